# revision 25
# baseline (speedup 1.0000x reference)
"""YOLO-v2 loss kernel for Trainium2 (8 NeuronCores, data-parallel over batch).

All matching logic (which depends only on gboxes/labels, 8KB of input) is
precomputed on the host into per-slot constants + gather offsets; the device
does every computation that touches pyolos:
  - dense conf term: sum sigmoid(conf)^2 over all 16x5x676 positions
    (3 activation passes with a per-row accumulate),
  - one indirect gather of 90 channels per matched slot (tx/ty/tw/th are
    gathered twice so lt and rb box corners compute in single [S,4] ops;
    the label-class logit is gathered as its own column),
  - a fused IoU/decode chain on DVE, off-chain terms on Pool,
  - a PE ones-matmul reduces the [128, 7] partials to one row, so the
    output DMA is a single packet; host does the all-reduce-mean.

Gathered column layout: [tx, ty, tx, ty, tw, th, tw, th, conf, cls_lbl,
cls0..cls79].  exp(-x) of cols 0:9 + reciprocal of (e + A9) gives
[sx, sy, sx, sy, e^tw, e^th, e^tw, e^th, pconf] in two ops.
"""

import numpy as np

from concourse import bass, mybir
from concourse.bass_utils import run_bass_kernel_spmd
from concourse.tile import TileContext

F32 = mybir.dt.float32
I32 = mybir.dt.int32
AF = mybir.ActivationFunctionType
OP = mybir.AluOpType
AX = mybir.AxisListType

NC = 8                 # cores
B = 128                # batch
BL = B // NC           # images per core (16)
NGT = 8                # GTs per image
S = BL * NGT           # slots per core (128)
GRID = 26
HW = GRID * GRID       # 676
NANC = 5
IMG = 425 * HW         # elements per image (287300)
PL = 5 * HW            # channel stride in elements (3380)
EPS = 1e-7
ANC = np.array([[0.05, 0.07], [0.12, 0.15], [0.25, 0.30],
                [0.45, 0.50], [0.80, 0.85]], np.float32)

# float consts layout (bit-packed into the combined i32 tensor after the
# gather offsets, so all per-slot host data rides ONE 128-packet DMA)
CF_A9 = 0       # [9] = [1,1,1,1,0,0,0,0,1]
CF_SC4 = 9      # [4] = [1/26, 1/26, -1/26, -1/26]
CF_ANCQ = 13    # [4] = [-aw/2, -ah/2, -aw/2, -ah/2]
CF_CRQ = 17     # [4] = [crx, cry, -crx, -cry] / 26
CF_GQ = 21      # [4] = [glx, gly, -grx, -gry]
CF_AREAA = 25   # [1] anchor area
CF_AGEPS = 26   # [1] gt area + eps
CF_ZM1 = 27     # [2] 1 - txy target
CF_TWHT = 29    # [2] twh target
CF_LW6 = 31     # [6] [lastw x4, lastw*weff x2]
CF_N = 37

NCHG = 90       # gathered: tx,ty,tx,ty,tw,th,tw,th,conf,cls_lbl,cls0..79
NCI = NCHG + CF_N


def _host_match(gbx: np.ndarray, lbl: np.ndarray):
    """Matching for one core's S slots. gbx [S,4] f32 ltrb, lbl [S] int.
    Mirrors reference.match_one in float32. Returns (cf [S,CF_N] f32,
    offs [S,NCHG] i32)."""
    gbx = gbx.astype(np.float32)
    cxy = (gbx[:, :2] + gbx[:, 2:]) * np.float32(0.5)
    wh = gbx[:, 2:] - gbx[:, :2]
    inter = np.minimum(wh[:, None, :], ANC[None]).prod(-1)
    areag = wh.prod(-1)
    areaa5 = (ANC[:, 0] * ANC[:, 1])
    iou2 = inter / (areag[:, None] + areaa5[None] - inter + np.float32(EPS))
    mign = iou2 > 0.5
    idxm = iou2.argmax(-1)
    colrow = (cxy * np.float32(GRID)).astype(np.int32)
    txy = (cxy - colrow.astype(np.float32) / np.float32(GRID)) * np.float32(GRID)
    twh = np.log(wh / ANC[idxm])
    weight = np.float32(2.0) - areag
    cell = colrow[:, 1] * GRID + colrow[:, 0]
    key = cell * NANC + idxm

    lastw = np.ones(S, np.float32)
    ign = np.zeros(S, np.float32)
    for i in range(BL):
        for a in range(NGT):
            s = i * NGT + a
            for j in range(a + 1, NGT):
                t = i * NGT + j
                if key[t] == key[s]:
                    lastw[s] = 0.0
                if cell[t] == cell[s] and mign[t, idxm[s]]:
                    ign[s] = 1.0
    weff = np.where(ign > 0, np.float32(-1.0), weight)

    cr26 = colrow.astype(np.float32) / np.float32(GRID)
    cf = np.zeros((S, CF_N), np.float32)
    cf[:, CF_A9:CF_A9 + 9] = [1, 1, 1, 1, 0, 0, 0, 0, 1]
    cf[:, CF_SC4:CF_SC4 + 4] = np.float32(1.0) / GRID * np.array([1, 1, -1, -1])
    ah = ANC[idxm] * np.float32(-0.5)
    cf[:, CF_ANCQ:CF_ANCQ + 4] = np.concatenate([ah, ah], 1)
    crq = np.concatenate([cr26, -cr26], 1)
    cf[:, CF_CRQ:CF_CRQ + 4] = crq
    # gq shifted by -crq: max(w4+crq, gq) = crq + max(w4, gq-crq) and the
    # crq halves cancel in niwh = im[0:2]+im[2:4], so pltn is never formed
    cf[:, CF_GQ:CF_GQ + 4] = (np.concatenate([gbx[:, 0:2], -gbx[:, 2:4]], 1)
                              - crq)
    cf[:, CF_AREAA] = areaa5[idxm]
    cf[:, CF_AGEPS] = areag + np.float32(EPS)
    cf[:, CF_ZM1:CF_ZM1 + 2] = np.float32(1.0) - txy
    cf[:, CF_TWHT:CF_TWHT + 2] = twh
    cf[:, CF_LW6:CF_LW6 + 4] = lastw[:, None]
    cf[:, CF_LW6 + 4:CF_LW6 + 6] = (lastw * weff)[:, None]

    img = np.arange(S) // NGT
    rowoff = img * IMG + idxm * HW + cell          # element offset of c=0
    chan = np.empty((S, NCHG), np.int64)
    chan[:, 0:8] = np.array([81, 82, 81, 82, 83, 84, 83, 84])[None, :]
    chan[:, 8] = 0
    chan[:, 9] = lbl.astype(np.int64)              # cls channel c = lbl
    chan[:, 10:90] = np.arange(1, 81)[None, :]
    offs = rowoff[:, None] + chan * PL
    ci = np.empty((S, NCI), np.int32)
    ci[:, 0:NCHG] = offs.astype(np.int32)
    ci[:, NCHG:] = cf.view(np.int32)
    return ci


def _split_multiwaits(nc: bass.Bass, k: int = 1) -> None:
    """This walrus build rejects instructions with >~2 sync waits; hoist
    extra waits onto preceding same-engine NoOps (equivalent for monotone
    sem-ge waits)."""
    for fn in nc.m.functions:
        for bb in fn.blocks:
            out = []
            for inst in bb.instructions:
                si = inst.sync_info
                waits = list(si.on_wait) if si is not None and si.on_wait else []
                if len(waits) > k:
                    for i, w in enumerate(waits[:-k]):
                        out.append(mybir.InstNoOp(
                            name=f"{inst.name}-wsplit{i}",
                            engine=inst.engine,
                            bass_nofuse=True,
                            sync_info=mybir.SyncInfo(on_wait=[w],
                                                     on_update=[]),
                        ))
                    inst.sync_info = mybir.SyncInfo(
                        on_wait=waits[-k:], on_update=list(si.on_update))
                out.append(inst)
            bb.instructions = out


def build_bass() -> bass.Bass:
    nc = bass.Bass()
    py = nc.declare_dram_parameter("pyolos", [BL, 425, HW], F32, isOutput=False)
    cip = nc.declare_dram_parameter("ci", [S, NCI], I32, isOutput=False)
    outp = nc.declare_dram_parameter("out", [1, 7], F32, isOutput=True)
    py_flat = py[:, :, :].rearrange("a b c -> (a b c)")

    with TileContext(nc) as tc:
        with tc.tile_pool(name="sb", bufs=1) as sb:
            # ---- tiles ----
            conf_t = sb.tile([BL * 5, HW], F32, name="conf_t")
            e80 = sb.tile([BL * 5, HW], F32, name="e80")
            pf = sb.tile([S, NCHG], F32, name="pf")
            ci_t = sb.tile([S, NCI], I32, name="ci_t")
            ones = sb.tile([S, 1], F32, name="ones")
            stack = sb.tile([S, 7], F32, name="stack")
            q = sb.tile([S, 6], F32, name="q")
            g2 = sb.tile([S, 2], F32, name="g2")

            def tt(shape, tag, dt=F32):
                return sb.tile(shape, dt, name=tag)

            # ---- DMA issues. The HW queues share the 16 DMA engines and
            # are packet-bound, so the small offsets+consts burst goes
            # first, conf's bulk second, both on one queue ----
            nc.sync.dma_start(out=ci_t[:], in_=cip[:, :])
            nc.sync.dma_start(out=conf_t[:], in_=py[:, 0:5, :])

            def ctf(a, b):
                return ci_t[:, NCHG + a:NCHG + b].bitcast(F32)

            # ---- Pool: memsets (no deps), then gather ----
            nc.gpsimd.memset(stack[64:S, 0:1], 0.0)  # rows 64:80 overwritten
            # by the dense accum below; partition starts must be mult. of 32
            nc.gpsimd.memset(q[:], 1.0)
            nc.gpsimd.memset(g2[:], 0.0)
            nc.gpsimd.memset(ones[:], 1.0)
            nc.gpsimd.indirect_dma_start(
                out=pf[:], out_offset=None,
                in_=py_flat.rearrange("(a b) -> a b", b=1),
                in_offset=bass.IndirectOffsetOnAxis(ap=ci_t[:, 0:NCHG],
                                                    axis=0))

            # ---- Scalar: dense conf chain + slot activations ----
            # sigma(x)^2 = exp(-2*softplus(-x)); Exp/Ln only -> one table set.
            nc.scalar.activation(e80[:], conf_t[:], AF.Exp, scale=-1.0)
            e9 = tt([S, 9], "e9")
            nc.scalar.activation(e9[:], pf[:, 0:9], AF.Exp, scale=-1.0)
            spn = tt([S, 2], "spn")
            nc.scalar.activation(spn[:], e9[:, 0:2], AF.Ln, bias=1.0)
            nc.scalar.activation(e80[:], e80[:], AF.Ln, bias=1.0)
            ecls = tt([S, 80], "ecls")
            nc.scalar.activation(ecls[:], pf[:, 10:90], AF.Exp)
            spsum = tt([S, 1], "spsum")
            nc.scalar.activation(ecls[:], ecls[:], AF.Ln, bias=1.0,
                                 accum_out=spsum[:])
            nc.scalar.activation(e80[:], e80[:], AF.Exp, scale=-2.0,
                                 accum_out=stack[0:BL * 5, 0:1])


            # ---- Pool: off-chain slot terms ----
            bb = tt([S, 2], "bb")
            nc.gpsimd.tensor_tensor(out=bb[:], in0=pf[:, 0:2],
                                    in1=ctf(CF_ZM1, CF_ZM1 + 2), op=OP.mult)
            bb2 = tt([S, 2], "bb2")
            nc.gpsimd.tensor_tensor(out=bb2[:], in0=bb[:], in1=spn[:],
                                    op=OP.add)
            nc.gpsimd.tensor_tensor(out=q[:, 4:5], in0=bb2[:, 0:1],
                                    in1=bb2[:, 1:2], op=OP.add)
            dwh = tt([S, 2], "dwh")
            nc.gpsimd.tensor_tensor(out=dwh[:], in0=pf[:, 4:6],
                                    in1=ctf(CF_TWHT, CF_TWHT + 2),
                                    op=OP.subtract)
            dw2 = tt([S, 2], "dw2")
            nc.gpsimd.tensor_tensor(out=dw2[:], in0=dwh[:], in1=dwh[:],
                                    op=OP.mult)
            nc.gpsimd.tensor_tensor(out=q[:, 5:6], in0=dw2[:, 0:1],
                                    in1=dw2[:, 1:2], op=OP.add)
            nc.gpsimd.tensor_tensor(out=q[:, 3:4], in0=spsum[:],
                                    in1=pf[:, 9:10], op=OP.subtract)

            # ---- DVE: IoU/decode chain ----
            c9 = tt([S, 9], "c9")
            nc.vector.tensor_tensor(out=c9[:], in0=e9[:],
                                    in1=ctf(CF_A9, CF_A9 + 9), op=OP.add)
            r9 = tt([S, 9], "r9")
            nc.vector.reciprocal(r9[:], c9[:])
            # r9 = [sx, sy, sx, sy, e^tw, e^th, e^tw, e^th, pconf]
            # pool computes the pred-area denominator pieces off-chain
            ea = tt([S, 1], "ea")
            nc.gpsimd.tensor_tensor(out=ea[:], in0=r9[:, 4:5],
                                    in1=r9[:, 5:6], op=OP.mult)
            eag = tt([S, 1], "eag")
            nc.gpsimd.tensor_tensor(out=eag[:], in0=ea[:],
                                    in1=ctf(CF_AREAA, CF_AREAA + 1),
                                    op=OP.mult)
            nc.gpsimd.tensor_tensor(out=eag[:], in0=eag[:],
                                    in1=ctf(CF_AGEPS, CF_AGEPS + 1),
                                    op=OP.add)
            # DVE chain continues
            v4 = tt([S, 4], "v4")
            nc.vector.tensor_tensor(out=v4[:], in0=r9[:, 0:4],
                                    in1=ctf(CF_SC4, CF_SC4 + 4), op=OP.mult)
            u4 = tt([S, 4], "u4")
            nc.vector.tensor_tensor(out=u4[:], in0=r9[:, 4:8],
                                    in1=ctf(CF_ANCQ, CF_ANCQ + 4),
                                    op=OP.mult)
            w4 = tt([S, 4], "w4")
            nc.vector.tensor_tensor(out=w4[:], in0=v4[:], in1=u4[:],
                                    op=OP.add)
            im4 = tt([S, 4], "im4")
            nc.vector.tensor_tensor(out=im4[:], in0=w4[:],
                                    in1=ctf(CF_GQ, CF_GQ + 4), op=OP.max)
            niwh = tt([S, 2], "niwh")
            nc.vector.tensor_tensor(out=niwh[:], in0=im4[:, 0:2],
                                    in1=im4[:, 2:4], op=OP.add)
            nc.vector.tensor_scalar(niwh[:], niwh[:], 0.0, None, OP.min)
            inter = tt([S, 1], "inter")
            nc.vector.tensor_tensor(out=inter[:], in0=niwh[:, 0:1],
                                    in1=niwh[:, 1:2], op=OP.mult)
            den = tt([S, 1], "den")
            nc.vector.tensor_tensor(out=den[:], in0=eag[:], in1=inter[:],
                                    op=OP.subtract)
            deni = tt([S, 1], "deni")
            nc.vector.reciprocal(deni[:], den[:])
            nc.vector.tensor_tensor(out=g2[:, 0:1], in0=inter[:],
                                    in1=deni[:], op=OP.mult)
            gpos = tt([S, 1], "gpos")
            nc.vector.tensor_scalar(gpos[:], g2[:, 0:1], 0.0, None, OP.is_gt)
            m6 = tt([S, 6], "m6")
            nc.vector.tensor_tensor(out=m6[:], in0=ctf(CF_LW6, CF_LW6 + 6),
                                    in1=gpos[:, 0:1].to_broadcast([S, 6]),
                                    op=OP.mult)
            d2 = tt([S, 2], "d2")
            nc.vector.tensor_tensor(out=d2[:],
                                    in0=r9[:, 8:9].to_broadcast([S, 2]),
                                    in1=g2[:], op=OP.subtract)
            nc.vector.tensor_tensor(out=q[:, 0:2], in0=d2[:], in1=d2[:],
                                    op=OP.mult)
            nc.vector.tensor_tensor(out=stack[:, 1:7], in0=q[:], in1=m6[:],
                                    op=OP.mult)

            # ---- output: reduce 128 rows on the idle PE, DMA one row ----
            with tc.tile_pool(name="ps", bufs=1, space="PSUM") as ps:
                red = ps.tile([1, 7], F32, name="red")
                nc.tensor.matmul(out=red[:], lhsT=ones[:], rhs=stack[:],
                                 start=True, stop=True)
                osb = sb.tile([1, 7], F32, name="osb")
                nc.vector.tensor_copy(osb[:], red[:])
                nc.sync.dma_start(out=outp[:, :], in_=osb[:])
    _split_multiwaits(nc, k=1)
    return nc


_NC_CACHE = None
LAST_RESULTS = None


def _get_nc():
    global _NC_CACHE
    if _NC_CACHE is None:
        _NC_CACHE = build_bass()
    return _NC_CACHE


def run(pyolos, gboxes_ltrb, labels, trace=False, **spmd_kwargs):
    global LAST_RESULTS
    nc = _get_nc()
    py = np.ascontiguousarray(
        np.asarray(pyolos, np.float32).reshape(B, 425, HW))
    gbx = np.asarray(gboxes_ltrb, np.float32)
    lbl = np.asarray(labels)
    in_maps = []
    for c in range(NC):
        sl = slice(c * BL, (c + 1) * BL)
        ci = _host_match(gbx[sl].reshape(S, 4), lbl[sl].reshape(S))
        in_maps.append({"pyolos": py[sl], "ci": ci})
    res = run_bass_kernel_spmd(nc, in_maps, list(range(NC)), trace=trace,
                               **spmd_kwargs)
    LAST_RESULTS = res
    t = np.zeros(7, np.float64)
    for r in res.results:
        t += r["out"].astype(np.float64).reshape(-1, 7).sum(0)
    dense_sq, pos_mse, pos_psq, npos, cls_num, txy_s, twh_s = t
    loss = (5.0 * pos_mse / B
            + (dense_sq - pos_psq) / B
            + cls_num / max(npos, 1.0)
            + txy_s / B
            + twh_s / B)
    return np.float32(loss)


def kernel(pyolos, gboxes_ltrb, labels):
    return run(pyolos, gboxes_ltrb, labels)
